# revision 5
# baseline (speedup 1.0000x reference)
"""Trainium2 Bass kernel for nn_LocationDependentClassifier.

Reference computation (for full input x of shape (64, 3, 512, 512) f32):
    top_left = x[:, :, :8, :8].mean(axis=(1, 2, 3))          # (64,)
    pred     = mod(trunc(top_left * 10), 10)                 # int in [0, 10)
    logits   = 10 * one_hot(pred, 10)                        # (64, 10) f32

Only the 8x8 top-left patch of each channel is live: 64*3*8*8 floats (48 KiB)
out of 201 MB. Sharding strategy (pure data parallelism per the hint): the
batch dim is split across the 8 cores, and each core is handed exactly the
bytes it needs -- its 8 images' top-left patches, flattened to (8, 192).

On-device per core (all fp32, all on the DVE; 4-op dependency chain):
    s = reduce_sum(patch_row_b)                              # (8, 1)
    S = (CONST <= s) * 10                                    # (8, 40)
    a = S[:, 0:20] - S[:, 20:40]                             # (8, 20)
    o = a[:, 0:10] + a[:, 10:20]                             # (8, 10)

CONST columns are [LO(20) | HI(20)], LO = [lo1 | lo2], HI = [hi1 | hi2], so
a[:, j] = 10 * ind(lo_j <= s < hi_j) and o sums the positive/negative trunc
branches. Class c fires iff t in [c, c+1) (positive branch; c=0 widens to
[-1, 1)) or t in [c-11, c-10) (negative branch, c >= 1), t = sum * 10/192.
Thresholds are pre-multiplied by 192/10 so the comparison runs on the raw
sum. Every intermediate is an exact small integer in fp32; the only
inexactness is the sum itself (boundary margin ~5 orders above fp32 noise).

This kernel is latency-bound: the NEFF boilerplate (engine-queue init,
DMA-queue doorbells, end-of-NEFF semaphore sweep) dominates the ~2 us of
real work. Beyond the minimal instruction count, the build below also:
  - skips Bass's const-AP memset preamble (unused by this kernel),
  - shrinks the declared DMA queues from 49 to 3 (fewer doorbell writes
    at execution start),
  - explicitly clears its semaphores at block end so the NEFF is
    re-executable without relying on the compiler's teardown sweep.
"""

import numpy as np

import concourse.bass as bass
import concourse.mybir as mybir
from concourse.bass_utils import run_bass_kernel_spmd
from concourse.tile import TileContext

B, C, H, W = 64, 3, 512, 512
PATCH = 8  # top-left patch is 8x8
NUM_CLASSES = 10
N_CORES = 8
PER_CORE = B // N_CORES  # 8 rows per core
D = C * PATCH * PATCH  # 192 reduced elements per row
SCALE = D / 10.0  # t = sum/SCALE; thresholds pre-multiplied by SCALE

_NC = None
LAST_RESULTS = None  # BassKernelResults of the most recent run (for test harness)


def _const_matrix() -> np.ndarray:
    """(PER_CORE, 4*NUM_CLASSES) f32: [LO1 | LO2 | HI1 | HI2] per class, in
    raw-sum units. Column j of the LO half pairs with column j of the HI
    half: out interval j = ind(LO_j <= sum < HI_j) * 10.
    """
    BIG = 1e30  # sentinel: comparison always false
    lo1 = np.array([-1.0] + [float(c) for c in range(1, NUM_CLASSES)])
    hi1 = np.array([float(c + 1) for c in range(NUM_CLASSES)])
    lo2 = np.array([BIG] + [float(c - 11) for c in range(1, NUM_CLASSES)])
    hi2 = np.array([BIG] + [float(c - 10) for c in range(1, NUM_CLASSES)])
    row = np.concatenate([lo1, lo2, hi1, hi2])
    row = np.where(np.abs(row) < 100.0, row * SCALE, row)
    return np.tile(row.astype(np.float32), (PER_CORE, 1))


def _build_nc() -> bass.Bass:
    # Raw Bass (no Tile): explicit semaphores, at most one sem wait per
    # instruction (CoreV2/V3 codegen rejects instructions that accumulate
    # several waits, which Tile's kernel-tail drain does for this shape of
    # kernel).
    #
    # Single input tensor per core: [x patch (192) | const matrix (40)] so
    # there is exactly one input DMA; the reduce takes the one cross-engine
    # wait and the remaining DVE ops rely on sem-guarded program order.
    #
    # Bass's __init__ preamble registers four const APs via gpsimd.memset;
    # nothing in this kernel reads them, so stub memset out for the
    # duration of construction to keep them off the pre-kernel barrier's
    # critical path. Likewise, only SP (DMA), DVE (compute) and Pool (the
    # barrier hub) execute anything here, so narrow every all-engine
    # barrier to those three engines -- PE/Activation never have to be
    # woken, synced, or swept.
    keep = [mybir.EngineType.Pool, mybir.EngineType.DVE, mybir.EngineType.SP]
    orig_memset = bass.BassGpSimd.memset
    orig_aeb = bass.Bass.all_engine_barrier
    bass.BassGpSimd.memset = lambda self, *a, **k: None
    bass.Bass.all_engine_barrier = (
        lambda self, sem_only=False: self.multi_engine_barrier(keep)
    )
    try:
        nc = bass.Bass(name="loc_cls")

        f32 = mybir.dt.float32
        W4 = 4 * NUM_CLASSES
        xp = nc.dram_tensor("xp", (PER_CORE, D + W4), f32, kind="ExternalInput")
        out = nc.dram_tensor(
            "out", (PER_CORE, NUM_CLASSES), f32, kind="ExternalOutput"
        )
        NC = NUM_CLASSES

        with (
            nc.sbuf_tensor([PER_CORE, D + W4], f32) as xt,
            nc.sbuf_tensor([PER_CORE, 1], f32) as s,
            nc.sbuf_tensor([PER_CORE, W4], f32) as S,
            nc.sbuf_tensor([PER_CORE, 2 * NC], f32) as a,
            nc.sbuf_tensor([PER_CORE, NC], f32) as o,
            nc.semaphore() as dma_sem,
            nc.semaphore() as vsem,
            nc.Block() as block,
        ):

            @block.sync
            def _(sync):
                sync.dma_start(out=xt[:], in_=xp[:]).then_inc(dma_sem, 16)
                sync.wait_ge(vsem, 4)
                sync.dma_start(out=out[:], in_=o[:]).then_inc(dma_sem, 16)
                sync.wait_ge(dma_sem, 32)
                # Leave every kernel semaphore at 0 so the NEFF re-executes
                # correctly regardless of the runtime's teardown behavior.
                sync.sem_clear(vsem)
                sync.sem_clear(dma_sem)

            @block.vector
            def _(vector):
                # The DVE is deeply pipelined: a dependent instruction issued
                # back-to-back reads stale data (CoreSim race detector
                # confirms). Every RAW edge below is guarded by a sem
                # inc/wait pair.
                vector.wait_ge(dma_sem, 16)
                vector.reduce_sum(
                    out=s[:], in_=xt[:, 0:D], axis=mybir.AxisListType.X
                ).then_inc(vsem, 1)
                vector.wait_ge(vsem, 1)
                # S = (cst <= sum) * 10  -- one fused compare+scale op
                vector.tensor_scalar(
                    out=S[:],
                    in0=xt[:, D : D + W4],
                    scalar1=s[:],
                    scalar2=10.0,
                    op0=mybir.AluOpType.is_le,
                    op1=mybir.AluOpType.mult,
                ).then_inc(vsem, 1)
                vector.wait_ge(vsem, 2)
                # a = 10*(ind(sum >= LO) - ind(sum >= HI)): interval one-hots
                vector.tensor_tensor(
                    out=a[:], in0=S[:, 0 : 2 * NC], in1=S[:, 2 * NC : 4 * NC],
                    op=mybir.AluOpType.subtract,
                ).then_inc(vsem, 1)
                vector.wait_ge(vsem, 3)
                # o = positive-branch + negative-branch interval indicators
                vector.tensor_tensor(
                    out=o[:], in0=a[:, 0:NC], in1=a[:, NC : 2 * NC],
                    op=mybir.AluOpType.add,
                ).then_inc(vsem, 1)
    finally:
        bass.BassGpSimd.memset = orig_memset
        bass.Bass.all_engine_barrier = orig_aeb

    # PE / Activation only carry dead preamble register-moves; drop them so
    # the compiled NEFF gives those engines nothing to do.
    drop = {mybir.EngineType.PE, mybir.EngineType.Activation}
    for func in nc.m.functions:
        for bb in func.blocks:
            bb.instructions = [i for i in bb.instructions if i.engine not in drop]

    # Declared DMA queues drive NRT's per-execution queue setup. Default is
    # 3 declarations x 16 queues = ~49 physical queues; this kernel issues
    # exactly two DMAs, both from SP. Keep Pool's SWDGE queue (its engine
    # preamble configures it) and 2 SP HWDGE queues.
    for q in nc.m.queues:
        if q.name == "qPoolDynamic":
            q.num_queues = 1
        elif q.name == "qSPDynamicHW":
            q.num_queues = 2
    nc.m.queues = [q for q in nc.m.queues if q.name != "qActDynamicHW"]

    return nc


def _get_nc() -> bass.Bass:
    global _NC
    if _NC is None:
        _NC = _build_nc()
    return _NC


def kernel(x: np.ndarray) -> np.ndarray:
    global LAST_RESULTS
    x = np.asarray(x)
    assert x.shape == (B, C, H, W), x.shape
    # Host-side sharding: slice out the only live bytes and split by batch.
    patch = x[:, :, :PATCH, :PATCH].astype(np.float32, copy=False).reshape(B, D)
    cst = _const_matrix()
    merged = np.concatenate([patch, np.tile(cst, (N_CORES, 1))], axis=1)
    in_maps = [
        {"xp": np.ascontiguousarray(merged[i * PER_CORE : (i + 1) * PER_CORE])}
        for i in range(N_CORES)
    ]
    res = run_bass_kernel_spmd(_get_nc(), in_maps, core_ids=list(range(N_CORES)))
    LAST_RESULTS = res
    return np.concatenate(
        [res.results[i]["out"] for i in range(N_CORES)], axis=0
    ).astype(np.float32, copy=False)
